# revision 26
# baseline (speedup 1.0000x reference)
"""v3 config snapshot (measured 170.6us once): 5-tap taper groups,
greedy queue balance (which happened to alternate rings), single xtile
split across both HWDGE queue heads, kpool bufs=3, tmp bufs=5."""

import numpy as np

B, C, H, W, K = 4, 32, 256, 256, 5
P = (K - 1) // 2
CP = 16
YG = 8
RG = H // YG
WP = W + 2 * P
SROWS = RG + 2 * P
SLEN = SROWS * WP
HR = RG // 2
XROWS = HR + 2 * P  # 20 rows per half-stripe (halo incl)
XLEN = XROWS * WP   # 5200 elems per partition half-stripe
HFREE = HR * W
HHALF = HFREE // 2
NBANK = HFREE // 512
GMAX = 5
GFREE = GMAX * HFREE

GROUPS = [
    (0, 0, 1), (0, 1, 2), (0, 3, 3), (0, 6, 4), (0, 10, 5), (0, 15, 5),
    (0, 20, 5),
    (1, 0, 5), (1, 5, 5), (1, 10, 5), (1, 15, 5), (1, 20, 3), (1, 23, 2),
]
KTOTAL = 128 * 2 * K * K * HFREE

_cache = {}


def _build_nc():
    import concourse.bass as bass
    import concourse.tile as tile
    from concourse import bacc, mybir

    f32 = mybir.dt.float32
    f16 = mybir.dt.float16
    nc = bacc.Bacc("TRN2", target_bir_lowering=False, debug=False, num_devices=8)

    xs_t = nc.dram_tensor("xs", [128, 2 * XLEN], f16, kind="ExternalInput")
    ks_t = nc.dram_tensor("ks", [KTOTAL], f16, kind="ExternalInput")
    ident_t = nc.dram_tensor("ident", [128, 128], f16, kind="ExternalInput")
    out_t = nc.dram_tensor("out", [128, 2 * HFREE], f16, kind="ExternalOutput")

    with tile.TileContext(nc) as tc:
        with (
            tc.tile_pool(name="xp", bufs=1) as xpool,
            tc.tile_pool(name="idp", bufs=1) as ipool,
            tc.tile_pool(name="kp", bufs=3) as kpool,
            tc.tile_pool(name="tp", bufs=7) as tpool,
            tc.tile_pool(name="op", bufs=2) as opool,
            tc.tile_pool(name="pp", bufs=1, space="PSUM") as ppool,
        ):
            # h=0 x rows split across BOTH ring heads (smallest possible
            # blocker for the first tap product); h=1 rows + ident ride
            # the idle gpsimd/SWDGE queue.  Kernel groups then strictly
            # alternate rings (FIFO per ring; alternation = in-order
            # delivery with both rings continuously busy).
            ident = ipool.tile([128, 128], f16)
            nc.gpsimd.dma_start(out=ident[:], in_=ident_t[:, :])

            xtA = xpool.tile([128, XLEN], f16, tag="xa")
            xtB = xpool.tile([128, XLEN], f16, tag="xb")
            XH = XLEN // 2
            nc.sync.dma_start(out=xtA[:, :XH], in_=xs_t[:, :XH])
            nc.scalar.dma_start(out=xtA[:, XH:], in_=xs_t[:, XH:XLEN])
            nc.gpsimd.dma_start(out=xtB[:], in_=xs_t[:, XLEN:])

            x3h = [
                xtA[:].rearrange("p (r w) -> p r w", w=WP),
                xtB[:].rearrange("p (r w) -> p r w", w=WP),
            ]

            off = 0
            for gi, (h, t0, nt) in enumerate(GROUPS):
                glen = nt * HFREE
                ktile = kpool.tile([128, GFREE], f16, tag="kt")
                ksrc = bass.AP(ks_t, off, [[glen, 128], [1, glen]])
                off += 128 * glen
                keng = nc.sync if gi % 2 == 0 else nc.scalar
                keng.dma_start(out=ktile[:, :glen], in_=ksrc)

                if t0 == 0:
                    ptile = ppool.tile([128, HFREE], f32, tag="ps")
                for t in range(nt):
                    ij = t0 + t
                    i, j = divmod(ij, K)
                    k3 = ktile[:, t * HFREE : (t + 1) * HFREE].rearrange(
                        "p (r w) -> p r w", w=W
                    )
                    xv = x3h[h][:, i : i + HR, j : j + W]
                    tmp = tpool.tile([128, HFREE], f16, tag="tmp")
                    t3 = tmp[:].rearrange("p (r w) -> p r w", w=W)
                    nc.vector.tensor_mul(t3, xv, k3)
                    for bk in range(NBANK):
                        nc.tensor.matmul(
                            out=ptile[:, bk * 512 : (bk + 1) * 512],
                            lhsT=ident[:],
                            rhs=tmp[:, bk * 512 : (bk + 1) * 512],
                            start=(ij == 0),
                            stop=(ij == K * K - 1),
                        )

                if t0 + nt == K * K:
                    for q in range(2):
                        ob = opool.tile([128, HHALF], f16, tag="ob")
                        src = ptile[:, q * HHALF : (q + 1) * HHALF]
                        if h == 1 and q == 1:
                            nc.vector.tensor_copy(ob[:], src)
                        else:
                            nc.scalar.copy(ob[:], src)
                        dst = bass.AP(
                            out_t,
                            h * HFREE + q * HHALF,
                            [[2 * HFREE, 128], [1, HHALF]],
                        )
                        if h == 0:
                            nc.gpsimd.dma_start(out=dst, in_=ob[:])
                        else:
                            seng = nc.sync if q == 0 else nc.scalar
                            seng.dma_start(out=dst, in_=ob[:])

    nc.compile()
    return nc


def _get_nc():
    if "nc" not in _cache:
        _cache["nc"] = _build_nc()
    return _cache["nc"]


_IDENT = np.eye(128, dtype=np.float16)
# part A = stripe rows 0..19 (h=0), part B = stripe rows 16..35 (h=1)
_ROWIDXA = (np.arange(YG)[:, None] * RG + np.arange(XROWS)[None, :])
_ROWIDXB = _ROWIDXA + HR


def _make_in_maps(x, kernel):
    x = np.asarray(x, dtype=np.float32).astype(np.float16)
    kern = np.asarray(kernel, dtype=np.float32).astype(np.float16)
    xpad = np.pad(x, ((0, 0), (0, 0), (P, P), (P, P)), mode="edge")

    in_maps = []
    for core in range(8):
        b, half = divmod(core, 2)
        c0 = half * CP
        xp_c = xpad[b, c0 : c0 + CP]
        xs = np.concatenate(
            [
                xp_c[:, _ROWIDXA, :].reshape(128, XLEN),
                xp_c[:, _ROWIDXB, :].reshape(128, XLEN),
            ],
            axis=1,
        )
        kc = kern[b, c0 * K * K : (c0 + CP) * K * K]
        kc = kc.reshape(CP, K * K, YG, 2, HR, W).transpose(3, 1, 0, 2, 4, 5)
        ks = np.empty(KTOTAL, dtype=np.float16)
        off = 0
        for h, t0, nt in GROUPS:
            reg = kc[h, t0 : t0 + nt].transpose(1, 2, 0, 3, 4)
            n = 128 * nt * HFREE
            ks[off : off + n] = reg.reshape(-1)
            off += n
        in_maps.append(
            {"xs": np.ascontiguousarray(xs), "ks": ks, "ident": _IDENT}
        )
    return in_maps


def kernel(x, kernel, kernel_size):
    from concourse.bass_utils import run_bass_kernel_spmd

    in_maps = _make_in_maps(x, kernel)
    nc = _get_nc()
    res = run_bass_kernel_spmd(nc, in_maps, list(range(8)))

    out = np.empty((B, C, H, W), dtype=np.float32)
    for core in range(8):
        b, half = divmod(core, 2)
        c0 = half * CP
        o = res.results[core]["out"].reshape(CP, YG, 2, HR, W)
        out[b, c0 : c0 + CP] = o.reshape(CP, H, W).astype(np.float32)
    return out


# revision 27
# speedup vs baseline: 1.2173x; 1.2173x over previous
"""v3 config snapshot (measured 170.6us once): 5-tap taper groups,
greedy queue balance (which happened to alternate rings), single xtile
split across both HWDGE queue heads, kpool bufs=3, tmp bufs=5."""

import numpy as np

B, C, H, W, K = 4, 32, 256, 256, 5
P = (K - 1) // 2
CP = 16
YG = 8
RG = H // YG
WP = W + 2 * P
SROWS = RG + 2 * P
SLEN = SROWS * WP
HR = RG // 2
HFREE = HR * W
HHALF = HFREE // 2
NBANK = HFREE // 512
GMAX = 5
GFREE = GMAX * HFREE

GROUPS = [
    (0, 0, 1), (0, 1, 2), (0, 3, 3), (0, 6, 4), (0, 10, 5), (0, 15, 5),
    (0, 20, 5),
    (1, 0, 5), (1, 5, 5), (1, 10, 5), (1, 15, 5), (1, 20, 3), (1, 23, 2),
]
KTOTAL = 128 * 2 * K * K * HFREE

_cache = {}


def _build_nc():
    import concourse.bass as bass
    import concourse.tile as tile
    from concourse import bacc, mybir

    f32 = mybir.dt.float32
    f16 = mybir.dt.float16
    nc = bacc.Bacc("TRN2", target_bir_lowering=False, debug=False, num_devices=8)

    xs_t = nc.dram_tensor("xs", [128, SLEN], f16, kind="ExternalInput")
    ks_t = nc.dram_tensor("ks", [KTOTAL], f16, kind="ExternalInput")
    ident_t = nc.dram_tensor("ident", [128, 128], f16, kind="ExternalInput")
    out_t = nc.dram_tensor("out", [128, 2 * HFREE], f16, kind="ExternalOutput")

    with tile.TileContext(nc) as tc:
        with (
            tc.tile_pool(name="xp", bufs=1) as xpool,
            tc.tile_pool(name="idp", bufs=1) as ipool,
            tc.tile_pool(name="kp", bufs=3) as kpool,
            tc.tile_pool(name="tp", bufs=5) as tpool,
            tc.tile_pool(name="op", bufs=2) as opool,
            tc.tile_pool(name="pp", bufs=1, space="PSUM") as ppool,
        ):
            xtile = xpool.tile([128, SLEN], f16)
            XH = SLEN // 2
            nc.sync.dma_start(out=xtile[:, :XH], in_=xs_t[:, :XH])
            nc.scalar.dma_start(out=xtile[:, XH:], in_=xs_t[:, XH:])

            ident = ipool.tile([128, 128], f16)
            nc.gpsimd.dma_start(out=ident[:], in_=ident_t[:, :])

            x3 = xtile[:].rearrange("p (r w) -> p r w", w=WP)

            qbytes = {"sync": SLEN // 2, "scalar": SLEN // 2}
            off = 0
            for h, t0, nt in GROUPS:
                glen = nt * HFREE
                ktile = kpool.tile([128, GFREE], f16, tag="kt")
                ksrc = bass.AP(ks_t, off, [[glen, 128], [1, glen]])
                off += 128 * glen
                qname = min(qbytes, key=qbytes.get)
                qbytes[qname] += glen
                keng = nc.sync if qname == "sync" else nc.scalar
                keng.dma_start(out=ktile[:, :glen], in_=ksrc)

                if t0 == 0:
                    ptile = ppool.tile([128, HFREE], f32, tag="ps")
                for t in range(nt):
                    ij = t0 + t
                    i, j = divmod(ij, K)
                    k3 = ktile[:, t * HFREE : (t + 1) * HFREE].rearrange(
                        "p (r w) -> p r w", w=W
                    )
                    r0 = h * HR
                    xv = x3[:, i + r0 : i + r0 + HR, j : j + W]
                    tmp = tpool.tile([128, HFREE], f16, tag="tmp")
                    t3 = tmp[:].rearrange("p (r w) -> p r w", w=W)
                    nc.vector.tensor_mul(t3, xv, k3)
                    for bk in range(NBANK):
                        nc.tensor.matmul(
                            out=ptile[:, bk * 512 : (bk + 1) * 512],
                            lhsT=ident[:],
                            rhs=tmp[:, bk * 512 : (bk + 1) * 512],
                            start=(ij == 0),
                            stop=(ij == K * K - 1),
                        )

                if t0 + nt == K * K:
                    for q in range(2):
                        ob = opool.tile([128, HHALF], f16, tag="ob")
                        src = ptile[:, q * HHALF : (q + 1) * HHALF]
                        if h == 1 and q == 1:
                            nc.vector.tensor_copy(ob[:], src)
                        else:
                            nc.scalar.copy(ob[:], src)
                        dst = bass.AP(
                            out_t,
                            h * HFREE + q * HHALF,
                            [[2 * HFREE, 128], [1, HHALF]],
                        )
                        if h == 0:
                            nc.gpsimd.dma_start(out=dst, in_=ob[:])
                        else:
                            seng = nc.sync if q == 0 else nc.scalar
                            seng.dma_start(out=dst, in_=ob[:])

    nc.compile()
    return nc


def _get_nc():
    if "nc" not in _cache:
        _cache["nc"] = _build_nc()
    return _cache["nc"]


_IDENT = np.eye(128, dtype=np.float16)
_ROWIDX = (np.arange(YG)[:, None] * RG + np.arange(SROWS)[None, :])


def _make_in_maps(x, kernel):
    x = np.asarray(x, dtype=np.float32).astype(np.float16)
    kern = np.asarray(kernel, dtype=np.float32).astype(np.float16)
    xpad = np.pad(x, ((0, 0), (0, 0), (P, P), (P, P)), mode="edge")

    in_maps = []
    for core in range(8):
        b, half = divmod(core, 2)
        c0 = half * CP
        xs = xpad[b, c0 : c0 + CP][:, _ROWIDX, :].reshape(128, SLEN)
        kc = kern[b, c0 * K * K : (c0 + CP) * K * K]
        kc = kc.reshape(CP, K * K, YG, 2, HR, W).transpose(3, 1, 0, 2, 4, 5)
        ks = np.empty(KTOTAL, dtype=np.float16)
        off = 0
        for h, t0, nt in GROUPS:
            reg = kc[h, t0 : t0 + nt].transpose(1, 2, 0, 3, 4)
            n = 128 * nt * HFREE
            ks[off : off + n] = reg.reshape(-1)
            off += n
        in_maps.append(
            {"xs": np.ascontiguousarray(xs), "ks": ks, "ident": _IDENT}
        )
    return in_maps


def kernel(x, kernel, kernel_size):
    from concourse.bass_utils import run_bass_kernel_spmd

    in_maps = _make_in_maps(x, kernel)
    nc = _get_nc()
    res = run_bass_kernel_spmd(nc, in_maps, list(range(8)))

    out = np.empty((B, C, H, W), dtype=np.float32)
    for core in range(8):
        b, half = divmod(core, 2)
        c0 = half * CP
        o = res.results[core]["out"].reshape(CP, YG, 2, HR, W)
        out[b, c0 : c0 + CP] = o.reshape(CP, H, W).astype(np.float32)
    return out
